# revision 8
# baseline (speedup 1.0000x reference)
"""DeepSeek-V2-style MLA attention layer on 8 Trainium2 NeuronCores.

Tensor-parallel over heads: 16 heads / 8 cores = 2 local heads per core.
Each core computes: q/kv projections (kv_a replicated, q_b/kv_b column
sharded), RMSNorm + interleaved RoPE, causal attention for its 2 heads,
and a row-parallel o_proj partial. Partials are summed on the host.

v2 notes:
  * Everything matmul-related runs in bf16 (1 cycle/row on the PE at
    full p-state, 1 cycle/row transposes, 2x DVE, half the DMA bytes).
    PSUM accumulation stays fp32; softmax stats and rmsnorm stats fp32.
  * All constants ship in ONE dram tensor laid out exactly as the SBUF
    tile wants them ([128, 22528] bf16), so a single DMA loads every
    weight with zero staging copies. Two operands per core total
    (hidden + consts): per-operand dispatch overhead through the
    tunnel dominates e2e iteration time.
  * Phase 1 fuses projection, rope, feature-major transposes and the
    kv_b up-projection per 512-token chunk so eviction work on
    Vector/Scalar/GpSimd overlaps TensorE instead of forming barriers.
  * Attention is software-pipelined two k-tiles ahead (scores ->
    exp/mask -> AV) and the per-(chunk, head) softmax normalization is
    deferred behind the next unit's score matmuls.
  * Causal mask = GpSimd affine_select on diagonal tiles after exp.
    Softmax skips max-subtraction (scores ~N(0,1) after scaling).
"""

import numpy as np

T = 2048
HID = 2048
H = 16
DN = 128   # qk nope dims
DR = 64    # qk rope dims
DV = 128   # v dims
KV = 512   # kv lora rank
EPS = 1e-6
THETA = 10000.0
SCALE = float((DN + DR) ** -0.5)
NCORES = 8
HL = H // NCORES          # local heads = 2
NT = T // 128             # 16 token tiles
NTCH = 4                  # 512-token chunks
WCOLS = HL * (DN + DR) + KV + DR   # 960 combined projection cols
NKB = HID // 128          # 16 contraction tiles over hidden dim
NLB = KV // 128           # 4 contraction tiles over latent dim

# column offsets inside the packed const tile [128, CONST_F]
OFF_W = 0                              # combined projection weights, 16 x 960
OFF_KBK = OFF_W + NKB * WCOLS          # 15360
OFF_KBV = OFF_KBK + NLB * HL * DN      # 16384
OFF_WO = OFF_KBV + NLB * HL * DV       # 17408
OFF_COS = OFF_WO + HL * HID            # 21504
OFF_SIN = OFF_COS + NT * (DR // 2)     # 22016
CONST_F = OFF_SIN + NT * (DR // 2)     # 22528

_CACHE = {}


def _split_sync_waits(nc, maxw=1):
    """This walrus build rejects instructions with more than one sync
    wait; hoist excess on_wait entries onto preceding same-engine NoOps."""
    import json
    import bass_rust

    bir = json.loads(nc.to_json_str())
    n = 0
    changed = 0
    for f in bir["functions"]:
        for blk in f["blocks"]:
            insts = blk.get("instructions")
            if not insts:
                continue
            out = []
            for inst in insts:
                si = inst.get("sync_info")
                ow = (si or {}).get("on_wait") or []
                if len(ow) > maxw and inst.get("engine") not in (None, "Unassigned"):
                    changed += 1
                    extra, keep = ow[:-maxw], ow[-maxw:]
                    inst["sync_info"]["on_wait"] = keep
                    for i in range(0, len(extra), maxw):
                        n += 1
                        out.append({
                            "debug": inst.get("debug", 0),
                            "engine": inst["engine"],
                            "ins": [],
                            "name": f"I-waitsplit-{n}",
                            "opcode": "NoOp",
                            "outs": [],
                            "text_hint": "waitsplit",
                            "sync_info": {"on_update": [],
                                          "on_wait": extra[i:i + maxw]},
                        })
                out.append(inst)
            blk["instructions"] = out
    if changed:
        nc.m = bass_rust.module_from_json_string(json.dumps(bir))


def _build_nc():
    import concourse.bass as bass
    import concourse.mybir as mybir
    import concourse.tile as tile
    from concourse.masks import make_identity

    f32 = mybir.dt.float32
    bf16 = mybir.dt.bfloat16
    ACT = mybir.ActivationFunctionType
    ALU = mybir.AluOpType
    AX = mybir.AxisListType

    nc = bass.Bass("TRN2", target_bir_lowering=False, debug=False,
                   num_devices=NCORES, enable_partition_id=False)

    hid_d = nc.dram_tensor("hidden", [T, HID], bf16, kind="ExternalInput")
    cst_d = nc.dram_tensor("consts", [128, CONST_F], bf16, kind="ExternalInput")
    out_d = nc.dram_tensor("out", [T, HID], bf16, kind="ExternalOutput")

    with tile.TileContext(nc) as tc:
        # ---------- persistent constants -------------------------------
        persist = tc.alloc_tile_pool(name="persist", bufs=1, side="left")
        cst = persist.tile([128, CONST_F], bf16)

        ident_b = persist.tile([128, 128], bf16)
        make_identity(nc, ident_b)
        ones_b = persist.tile([128, 1], bf16)
        nc.vector.memset(ones_b[:], 1.0)
        ones1_b = persist.tile([1, 128], bf16)
        nc.vector.memset(ones1_b[:], 1.0)
        eps_t = persist.tile([128, 1], f32)
        nc.vector.memset(eps_t[:], EPS)
        one_f = persist.tile([1, 1], f32)
        nc.vector.memset(one_f[:], 1.0)

        cos_t = cst[:, OFF_COS:OFF_SIN].rearrange("p (n f) -> p n f", f=DR // 2)
        sin_t = cst[:, OFF_SIN:CONST_F].rearrange("p (n f) -> p n f", f=DR // 2)

        # ---------- persistent attention operands (feature-major) ------
        pL = tc.alloc_tile_pool(name="attn_ops", bufs=1, side="left")
        qnT = pL.tile([128, HL, NT, 128], bf16, tag="qnT")
        qpT = pL.tile([64, HL, NT, 128], bf16, tag="qpT")
        kpT = pL.tile([64, NT, 128], bf16, tag="kpT")
        knT = pL.tile([128, HL, NT, 128], bf16, tag="knT")
        v_tok = pL.tile([128, NT, HL * DV], bf16, tag="v_tok")
        attnT = pL.tile([128, HL, NT, 128], bf16, tag="attnT")

        # ---------- phase-1 rotating pools (right stack, LIFO) ----------
        hrt_p = tc.alloc_tile_pool(name="hrt", bufs=7, side="right")
        qnp_p = tc.alloc_tile_pool(name="qnp", bufs=2, side="right")
        kpe_p = tc.alloc_tile_pool(name="kpe", bufs=2, side="right")
        kvc_p = tc.alloc_tile_pool(name="kvc", bufs=2, side="right")
        rot_p = tc.alloc_tile_pool(name="rot", bufs=2, side="right")
        rtmp_p = tc.alloc_tile_pool(name="rtmp", bufs=2, side="right")
        kvcT_p = tc.alloc_tile_pool(name="kvcT", bufs=2, side="right")
        st_p = tc.alloc_tile_pool(name="stats", bufs=2, side="right")
        # PSUM: ps_tr 2x1 + ps_proj 2x2 + ps_mid 2x1 = 8 banks
        ps_tr = tc.alloc_tile_pool(name="ps_tr", bufs=2, space="PSUM")
        ps_proj = tc.alloc_tile_pool(name="ps_proj", bufs=2, space="PSUM")
        ps_mid = tc.alloc_tile_pool(name="ps_mid", bufs=2, space="PSUM")

        CKV0 = HL * (DN + DR)          # 384: kv latent col offset

        # prefetch the first chunk's hidden slabs interleaved with the
        # projection weights so the PE starts within ~4us of kernel start
        hrt_pre = {}

        def want_hrt(t):
            # rolling prefetch of XBAR-transposed hidden tiles. All on the
            # Activation HWDGE queue: the loads have no waits (cannot stall
            # scalar compute), and XBAR transposes must stay on ONE queue -
            # running them concurrently from both queues corrupts data.
            if t < NT and t not in hrt_pre:
                s = hrt_pre[t] = hrt_p.tile([128, NKB, 128], bf16,
                                            name="hrt")
                nc.scalar.dma_start(out=s[:],
                                    in_=hid_d[t * 128:(t + 1) * 128, :],
                                    transpose=True)

        for ti in range(6):
            want_hrt(ti)
            if ti < 4:
                nc.sync.dma_start(out=cst[:, ti * 3840:(ti + 1) * 3840],
                                  in_=cst_d[:, ti * 3840:(ti + 1) * 3840])
        nc.sync.dma_start(out=cst[:, OFF_KBK:CONST_F],
                          in_=cst_d[:, OFF_KBK:CONST_F])

        def rope(src, dst, nt4, eng):
            # src/dst free dims [4, DR] with [even|odd] halves; nt4 =
            # (cos, sin) slices [128, 4, 32]
            half = DR // 2
            ev, od = src[:, :, 0:half], src[:, :, half:DR]
            t1 = rtmp_p.tile([128, 2, half], bf16, name="t1", tag="t1")
            t2 = rtmp_p.tile([128, 2, half], bf16, name="t2", tag="t2")
            eng.tensor_tensor(t1[:], ev, nt4[0], op=ALU.mult)
            eng.tensor_tensor(t2[:], od, nt4[1], op=ALU.mult)
            eng.tensor_tensor(dst[:, :, 0:half], t1[:], t2[:], op=ALU.subtract)
            eng.tensor_tensor(t1[:], od, nt4[0], op=ALU.mult)
            eng.tensor_tensor(t2[:], ev, nt4[1], op=ALU.mult)
            eng.tensor_tensor(dst[:, :, half:DR], t1[:], t2[:], op=ALU.add)

        for tch in range(NTCH):
            t0 = 4 * tch
            qnp = qnp_p.tile([128, 4, CKV0], bf16, name="qnp")
            kpe = kpe_p.tile([128, 4, DR], bf16, name="kpe")
            kvc = kvc_p.tile([128, 4, KV], bf16, name="kvc")
            q_rot = rot_p.tile([128, 4, HL * DR], bf16, name="q_rot", tag="qr")
            k_rot = rot_p.tile([128, 4, DR], bf16, name="k_rot", tag="kr")
            kvcT = kvcT_p.tile([128, NLB, 4, 128], bf16, name="kvcT")
            ssum = st_p.tile([128, 4, 1], f32, name="ssum", tag="ssum")
            srt = st_p.tile([128, 4, 1], f32, name="srt", tag="srt")
            rinv = st_p.tile([128, 4, 1], f32, name="rinv", tag="rinv")

            # ---- combined projection for 4 token tiles ----------------
            for j in range(4):
                ti = t0 + j
                want_hrt(ti + 3)
                hrt = hrt_pre.pop(ti)
                acc = ps_proj.tile([128, WCOLS], f32, name="acc")
                for hi in range(NKB):
                    w0 = cst[:, OFF_W + hi * WCOLS:OFF_W + hi * WCOLS + 512]
                    w1 = cst[:, OFF_W + hi * WCOLS + 512:
                             OFF_W + (hi + 1) * WCOLS]
                    nc.tensor.matmul(acc[:, 0:512], hrt[:, hi, :], w0,
                                     start=(hi == 0), stop=(hi == NKB - 1))
                    nc.tensor.matmul(acc[:, 512:WCOLS], hrt[:, hi, :], w1,
                                     start=(hi == 0), stop=(hi == NKB - 1))

                # evictions + fused rmsnorm of the kv latent slice
                nc.vector.tensor_copy(qnp[:, j, :], acc[:, 0:CKV0])
                nc.scalar.activation(kpe[:, j, :], acc[:, CKV0 + KV:WCOLS],
                                     ACT.Identity)
                sq = rtmp_p.tile([128, KV], f32, name="sq", tag="sq")
                nc.scalar.activation(sq[:], acc[:, CKV0:CKV0 + KV], ACT.Square)
                nc.vector.reduce_sum(ssum[:, j, :], sq[:], AX.X)
                nc.scalar.activation(srt[:, j, :], ssum[:, j, :], ACT.Sqrt,
                                     scale=1.0 / KV, bias=eps_t[:])
                nc.vector.reciprocal(rinv[:, j, :], srt[:, j, :])
                nc.scalar.activation(kvc[:, j, :], acc[:, CKV0:CKV0 + KV],
                                     ACT.Identity, scale=rinv[:, j, :])
                if j % 2 == 1:
                    # rope this pair of token tiles while the next pair's
                    # projection matmuls run
                    jj = slice(j - 1, j + 1)
                    ntp = (cos_t[:, t0 + j - 1:t0 + j + 1, :],
                           sin_t[:, t0 + j - 1:t0 + j + 1, :])
                    for h in range(HL):
                        qeng = nc.vector if h == 0 else nc.gpsimd
                        rope(qnp[:, jj, HL * DN + h * DR:
                                 HL * DN + (h + 1) * DR],
                             q_rot[:, jj, h * DR:(h + 1) * DR], ntp, qeng)
                    rope(kpe[:, jj, :], k_rot[:, jj, :], ntp,
                         nc.vector if j == 1 else nc.gpsimd)


            # ---- feature-major transposes + kv_b, interleaved so the
            # second rope half (j=2,3) finishes behind D(0,1)+v(pair0) ----
            def kvb_v(jp):
                va = ps_mid.tile([128, 2, HL * DV], f32, name="va", tag="mid")
                for j2 in range(2):
                    j = 2 * jp + j2
                    for lb in range(NLB):
                        nc.tensor.matmul(
                            va[:, j2, :],
                            kvcT[:, lb, j, :],
                            cst[:, OFF_KBV + lb * HL * DV:
                                OFF_KBV + (lb + 1) * HL * DV],
                            start=(lb == 0), stop=(lb == NLB - 1))
                nc.scalar.activation(v_tok[:, t0 + 2 * jp:t0 + 2 * jp + 2, :],
                                     va[:], ACT.Identity)

            def kvb_k(h):
                ka = ps_mid.tile([128, 512], f32, name="ka", tag="mid")
                for lb in range(NLB):
                    nc.tensor.matmul(
                        ka[:],
                        cst[:, OFF_KBK + lb * HL * DN + h * DN:
                            OFF_KBK + lb * HL * DN + (h + 1) * DN],
                        kvcT[:, lb, :, :],
                        start=(lb == 0), stop=(lb == NLB - 1))
                nc.vector.tensor_copy(knT[:, h, t0:t0 + 4, :], ka[:])

            def dtrans(j):
                ti = t0 + j
                pq = ps_tr.tile([128, 4, 128], bf16, name="pq", tag="tr")
                nc.tensor.transpose(pq[:, 0, :], qnp[:, j, 0:DN], ident_b[:])
                nc.tensor.transpose(pq[:, 1, :], qnp[:, j, DN:2 * DN],
                                    ident_b[:])
                pkv = ps_tr.tile([128, 4, 128], bf16, name="pkv", tag="tr")
                for lb in range(NLB):
                    nc.tensor.transpose(pkv[:, lb, :],
                                        kvc[:, j, lb * 128:(lb + 1) * 128],
                                        ident_b[:])
                nc.tensor.transpose(pq[:64, 2, :], q_rot[:, j, 0:DR],
                                    ident_b[:])
                nc.tensor.transpose(pq[:64, 3, :], q_rot[:, j, DR:2 * DR],
                                    ident_b[:])
                nc.tensor.transpose(pq[64:128, 2, :], k_rot[:, j, :],
                                    ident_b[:])
                nc.vector.tensor_copy(qnT[:, 0, ti, :], pq[:, 0, :])
                nc.vector.tensor_copy(qnT[:, 1, ti, :], pq[:, 1, :])
                nc.scalar.activation(kvcT[:, :, j, :], pkv[:], ACT.Identity)
                nc.vector.tensor_copy(qpT[:, :, ti, :], pq[:64, 2:4, :])
                nc.vector.tensor_copy(kpT[:, ti, :], pq[64:128, 2, :])

            dtrans(0)
            dtrans(1)
            kvb_v(0)
            dtrans(2)
            dtrans(3)
            kvb_v(1)
            kvb_k(0)
            kvb_k(1)

        # ---- release phase-1 pools (LIFO) -----------------------------
        ps_mid.release()
        ps_proj.release()
        ps_tr.release()
        st_p.release()
        kvcT_p.release()
        rtmp_p.release()
        rot_p.release()
        kvc_p.release()
        kpe_p.release()
        qnp_p.release()
        hrt_p.release()

        # ---------- phase 2: attention + o_proj -------------------------
        pt_p = tc.alloc_tile_pool(name="pT", bufs=4, side="right")
        ele_p = tc.alloc_tile_pool(name="ele", bufs=2, side="right")
        lt_p = tc.alloc_tile_pool(name="linvT", bufs=4, side="right")
        tmp_p = tc.alloc_tile_pool(name="otmp", bufs=3, side="right")
        osb_p = tc.alloc_tile_pool(name="osb", bufs=2, side="right")
        # PSUM: sT 2 + at 2 + el 1 + o 3 = 8 banks
        ps_sT = tc.alloc_tile_pool(name="ps_sT", bufs=2, space="PSUM")
        ps_at = tc.alloc_tile_pool(name="ps_at", bufs=2, space="PSUM")
        ps_el = tc.alloc_tile_pool(name="ps_el", bufs=1, space="PSUM")
        ps_o = tc.alloc_tile_pool(name="ps_o", bufs=3, space="PSUM")

        def norm_unit(qc, h, el_acc, linvT):
            # denominator reciprocal for one (q-chunk, head) unit, computed
            # transposed ([128 tok, 4 tile] -> all DVE lanes) and emitted
            # late so it overlaps the next unit's score matmuls
            def emit():
                ele = ele_p.tile([1, 512], f32, name="ele")
                nc.vector.tensor_copy(ele[:], el_acc[:])
                elT = ps_el.tile([128, 4], f32, name="el_acc")
                for b in range(4):
                    nc.tensor.transpose(elT[:, b:b + 1],
                                        ele[0:1, b * 128:(b + 1) * 128],
                                        one_f[:])
                nc.vector.reciprocal(linvT[:], elT[:])
            return emit

        pending = None
        linvTs = {}
        for qc in range(NTCH):
            nk = 4 * (qc + 1)
            qs = slice(4 * qc, 4 * qc + 4)
            for h in range(HL):
                at_acc = ps_at.tile([128, 512], f32, name="at_acc")
                el_acc = ps_el.tile([1, 512], f32, name="el_acc")
                linvT = linvTs[(qc, h)] = lt_p.tile([128, 4], f32,
                                                    name="linvT")
                pts = {}

                def scores(kt):
                    sT = ps_sT.tile([128, 512], f32, name="sT")
                    nc.tensor.matmul(sT[:], knT[:, h, kt, :], qnT[:, h, qs, :],
                                     start=True, stop=False)
                    nc.tensor.matmul(sT[:], kpT[:, kt, :], qpT[:, h, qs, :],
                                     start=False, stop=True)
                    pT = pt_p.tile([128, 512], bf16, name="pT")
                    nc.scalar.activation(pT[:], sT[:], ACT.Exp, scale=SCALE)
                    m = kt - 4 * qc
                    if m >= 0:
                        # keep where (512qc + qf) >= (128kt + p)
                        nc.gpsimd.affine_select(
                            out=pT[:], in_=pT[:], compare_op=ALU.is_ge,
                            fill=0.0, base=-128 * m, pattern=[[1, 512]],
                            channel_multiplier=-1)
                    pts[kt] = pT

                def accum(kt):
                    pT = pts.pop(kt)
                    nc.tensor.matmul(at_acc[:],
                                     v_tok[:, kt, h * DV:(h + 1) * DV],
                                     pT[:], start=(kt == 0),
                                     stop=(kt == nk - 1))
                    nc.tensor.matmul(el_acc[:], ones_b[:], pT[:],
                                     start=(kt == 0), stop=(kt == nk - 1))

                scores(0)
                scores(1)
                scores(2)
                for kt in range(3, nk):
                    scores(kt)
                    accum(kt - 3)
                    if kt == 3 and pending is not None:
                        pending()
                        pending = None
                for kt in range(nk - 3, nk):
                    accum(kt)
                if pending is not None:
                    pending()
                    pending = None
                nc.vector.tensor_copy(attnT[:, h, qs, :], at_acc[:])
                pending = norm_unit(qc, h, el_acc, linvT)

            # o_proj for the token tiles this q-chunk completed
            if pending is not None:
                pending()
                pending = None
            l0 = linvTs.pop((qc, 0))
            l1 = linvTs.pop((qc, 1))
            for j in range(4):
                ti = 4 * qc + j
                osb = osb_p.tile([128, HID], bf16, name="osb")
                for nch in range(HID // 512):
                    oa0 = ps_o.tile([128, 512], f32, name="oacc", tag="o")
                    nc.tensor.matmul(
                        oa0[:], attnT[:, 0, ti, :],
                        cst[:, OFF_WO + nch * 512:OFF_WO + (nch + 1) * 512],
                        start=True, stop=True)
                    oa1 = ps_o.tile([128, 512], f32, name="oacc", tag="o")
                    nc.tensor.matmul(
                        oa1[:], attnT[:, 1, ti, :],
                        cst[:, OFF_WO + HID + nch * 512:
                            OFF_WO + HID + (nch + 1) * 512],
                        start=True, stop=True)
                    ta = tmp_p.tile([128, 512], bf16, name="ta", tag="ta")
                    nc.scalar.activation(ta[:], oa0[:], ACT.Identity,
                                         scale=l0[:, j:j + 1])
                    tb = tmp_p.tile([128, 512], bf16, name="tb", tag="tb")
                    nc.vector.tensor_scalar(tb[:], oa1[:], l1[:, j:j + 1],
                                            None, ALU.mult)
                    nc.gpsimd.tensor_tensor(
                        osb[:, nch * 512:(nch + 1) * 512], ta[:], tb[:],
                        op=ALU.add)
                nc.sync.dma_start(
                    out=out_d[ti * 128:(ti + 1) * 128, :], in_=osb[:])

        ps_o.release()
        ps_el.release()
        ps_at.release()
        ps_sT.release()
        osb_p.release()
        tmp_p.release()
        lt_p.release()
        ele_p.release()
        pt_p.release()
        pL.release()
        persist.release()

    _split_sync_waits(nc)
    return nc


def _get_runner():
    if "run" in _CACHE:
        return _CACHE["run"]
    import jax
    from jax.experimental.shard_map import shard_map
    from jax.sharding import Mesh, PartitionSpec

    import concourse.mybir as mybir
    from concourse import bass2jax

    nc = _build_nc()
    bass2jax.install_neuronx_cc_hook()

    in_names, out_names, out_avals = [], [], []
    for alloc in nc.m.functions[0].allocations:
        if not isinstance(alloc, mybir.MemoryLocationSet):
            continue
        name = alloc.memorylocations[0].name
        if alloc.kind == "ExternalInput":
            in_names.append(name)
        elif alloc.kind == "ExternalOutput":
            out_names.append(name)
            shape = tuple(alloc.tensor_shape)
            dtype = mybir.dt.np(alloc.dtype)
            out_avals.append(jax.core.ShapedArray(shape, dtype))
    all_names = list(in_names)

    def _body(*args):
        outs = bass2jax._bass_exec_p.bind(
            *args,
            out_avals=tuple(out_avals),
            in_names=tuple(all_names),
            out_names=tuple(out_names),
            lowering_input_output_aliases=(),
            sim_require_finite=True,
            sim_require_nnan=True,
            nc=nc,
        )
        return tuple(outs)

    devices = jax.devices()[:NCORES]
    mesh = Mesh(np.asarray(devices), ("core",))
    sharded = jax.jit(
        shard_map(_body, mesh=mesh,
                  in_specs=(PartitionSpec("core"),) * len(in_names),
                  out_specs=(PartitionSpec("core"),) * len(out_names),
                  check_rep=False),
        keep_unused=True,
    )

    def run(in_maps):
        concat_in = [
            np.concatenate([np.asarray(m[name]) for m in in_maps], axis=0)
            for name in in_names
        ]
        out_arrs = sharded(*concat_in)
        jax.block_until_ready(out_arrs)
        results = []
        for c in range(NCORES):
            results.append({
                name: np.asarray(arr[c * arr.shape[0] // NCORES:
                                     (c + 1) * arr.shape[0] // NCORES])
                for name, arr in zip(out_names, out_arrs)
            })
        return results

    def make_timed(in_maps):
        from jax.sharding import NamedSharding
        sh = NamedSharding(mesh, PartitionSpec("core"))
        dev_in = [
            jax.device_put(
                np.concatenate([np.asarray(m[name]) for m in in_maps],
                               axis=0), sh)
            for name in in_names
        ]
        jax.block_until_ready(dev_in)

        def step():
            return sharded(*dev_in)

        return step

    _CACHE["run"] = run
    _CACHE["make_timed"] = make_timed
    return run


def _host_prep(positions, hidden_states, w_q, w_kv_a, kv_a_ln_w, w_kv_b, w_o):
    import ml_dtypes
    bf16 = ml_dtypes.bfloat16

    pos = np.asarray(positions).astype(np.float32)
    inv_freq = (1.0 / np.power(np.float32(THETA),
                               np.arange(0, DR, 2, dtype=np.float32)
                               / np.float32(DR))).astype(np.float32)
    freqs = pos[:, None] * inv_freq[None, :]
    cos_t = np.cos(freqs)
    sin_t = np.sin(freqs)

    def pack_pmajor(w, nblk):
        # [nblk*128, F] -> [128, nblk*F] partition-major
        f = w.shape[1]
        return np.ascontiguousarray(
            w.reshape(nblk, 128, f).transpose(1, 0, 2).reshape(128, nblk * f))

    hidden = np.asarray(hidden_states, dtype=np.float32).astype(bf16)
    w_q = np.asarray(w_q, dtype=np.float32)
    w_kv_a = np.asarray(w_kv_a, dtype=np.float32)
    w_kv_b_eff = np.asarray(kv_a_ln_w, dtype=np.float32)[:, None] * \
        np.asarray(w_kv_b, dtype=np.float32)
    w_o = np.asarray(w_o, dtype=np.float32)

    # reorder rope pair columns to [even | odd] halves
    perm = np.concatenate([np.arange(0, DR, 2), np.arange(1, DR, 2)])

    in_maps = []
    for c in range(NCORES):
        hs = [c * HL + h for h in range(HL)]
        qcols = [w_q[:, h * (DN + DR):h * (DN + DR) + DN] for h in hs]
        pcols = [w_q[:, h * (DN + DR) + DN:(h + 1) * (DN + DR)][:, perm]
                 for h in hs]
        kva_pe = w_kv_a[:, KV:][:, perm]
        w_comb = np.concatenate(qcols + pcols + [w_kv_a[:, :KV], kva_pe],
                                axis=1)
        wkb_k = np.concatenate(
            [w_kv_b_eff[:, h * (DN + DV):h * (DN + DV) + DN] for h in hs],
            axis=1)
        wkb_v = np.concatenate(
            [w_kv_b_eff[:, h * (DN + DV) + DN:(h + 1) * (DN + DV)]
             for h in hs], axis=1)
        wo_c = w_o[c * HL * DV:(c + 1) * HL * DV, :]
        consts = np.concatenate([
            pack_pmajor(w_comb, NKB),
            pack_pmajor(wkb_k, NLB),
            pack_pmajor(wkb_v, NLB),
            pack_pmajor(wo_c, HL),
            pack_pmajor(cos_t, NT),
            pack_pmajor(sin_t, NT),
        ], axis=1).astype(bf16)
        assert consts.shape == (128, CONST_F), consts.shape
        in_maps.append({"hidden": hidden, "consts": consts})
    return in_maps


def kernel(positions, hidden_states, w_q, w_kv_a, kv_a_ln_w, w_kv_b, w_o):
    in_maps = _host_prep(positions, hidden_states, w_q, w_kv_a, kv_a_ln_w,
                         w_kv_b, w_o)
    run = _get_runner()
    results = run(in_maps)
    out = results[0]["out"].astype(np.float32)
    for c in range(1, NCORES):
        out = out + results[c]["out"].astype(np.float32)
    return out.astype(np.float32)


if __name__ == "__main__":
    rng = np.random.default_rng(0)
    ins = {
        "positions": np.arange(T, dtype=np.int32),
        "hidden_states": rng.standard_normal((T, HID), dtype=np.float32),
        "w_q": rng.standard_normal((HID, H * (DN + DR)), dtype=np.float32)
        / np.sqrt(HID),
        "w_kv_a": rng.standard_normal((HID, KV + DR), dtype=np.float32)
        / np.sqrt(HID),
        "kv_a_ln_w": np.ones(KV, dtype=np.float32),
        "w_kv_b": rng.standard_normal((KV, H * (DN + DV)), dtype=np.float32)
        / np.sqrt(KV),
        "w_o": rng.standard_normal((H * DV, HID), dtype=np.float32)
        / np.sqrt(H * DV),
    }
    out = kernel(**ins)
    print("out", out.shape, out.dtype, float(np.abs(out).max()))
